# revision 1
# baseline (speedup 1.0000x reference)
"""Trainium2 Bass kernel for sheaf Dirichlet energy (ConsistencyBasedLaplacianBuilder).

loss = sum_e || maps[rev(e)] @ x[tgt(e)] - maps[e] @ x[src(e)] ||_F^2

Strategy (edge parallelism across 8 NeuronCores):
  The reference edge set is symmetric: edge e < H (=E/2) has its reverse at
  e + H, so the loss equals 2 * sum_{e<H} ||maps[e+H] x[dst] - maps[e] x[src]||^2.
  Each core takes a contiguous slice of the H half-edges, keeps a full replica
  of x in HBM, gathers x rows via indirect DMA (128 edges per tile, one edge
  per partition), and contracts on the vector engine with three wide ops per
  tile:
    prod[e, i, jj, f] = mcat[e, i, jj] * xcat[e, jj, f]      (f broadcast)
    diff[e, (i f)]    = sum_jj prod[e, i, jj, f]             (strided reduce)
    acc[e, tile]      = sum(diff * diff)                     (fused square+sum)
  where xcat = [x_dst | x_src] (jj in 0..7) and mcat interleaves maps_hi with
  negated maps_lo so the jj-sum forms the difference directly.
  Per-core partial sums are added on the host.
"""

import sys
import types

import numpy as np

sys.path.insert(0, "/opt/trn_rl_repo")

N = 50000
D = 4
F = 16
DF = D * F            # 64 floats per node row
E = 1600000
H = E // 2            # 800000 undirected pairs
NCORES = 8
EPC = H // NCORES     # 100000 half-edges per core

GROUP = 8             # tiles gathered per dma_gather pair
PAIR = 2 * GROUP      # tiles per loop iteration (double-buffered)
NT_USED = 800         # tiles per core (800*128 = 102400 >= 100000)
NT_ALLOC = 832        # padded columns (overhang gather reads into padding)
EPC_PAD = NT_USED * 128
# x is gathered with int16 indices (dma_gather), so it is split into two
# tables of XSPLIT+1 rows; row XSPLIT of each table is zero (out-of-range
# slot), and the two gathered halves are added.
XSPLIT = 25000


def _inject_axon_hooks():
    """The container's antenv lacks axon_hooks; provide it so NTFF tracing
    (used by test.py, harmless otherwise) can register."""
    if "antenv.axon_hooks" in sys.modules:
        return
    mod = types.ModuleType("antenv.axon_hooks")
    mod._hook = None

    def set_axon_ntff_profile_hook(h):
        mod._hook = h

    def get_axon_ntff_profile_hook():
        return mod._hook

    mod.set_axon_ntff_profile_hook = set_axon_ntff_profile_hook
    mod.get_axon_ntff_profile_hook = get_axon_ntff_profile_hook
    sys.modules["antenv.axon_hooks"] = mod


def _build_program(nt_used=NT_USED, nt_alloc=NT_ALLOC, n_nodes=N, ncores=NCORES):
    import concourse.bacc as bacc
    import concourse.bass as bass
    import concourse.tile as tile
    from concourse import mybir

    AP = bass.AP
    f32 = mybir.dt.float32
    i32 = mybir.dt.int32
    Op = mybir.AluOpType
    ds = bass.ds

    ngroups = nt_used // GROUP
    assert ngroups % 2 == 0
    niters = ngroups // 2

    i16 = mybir.dt.int16

    nc = bacc.Bacc("TRN2", target_bir_lowering=False, debug=False,
                   num_devices=ncores)

    xlo_d = nc.dram_tensor("xlo", [XSPLIT + 1, DF], f32, kind="ExternalInput")
    xhi_d = nc.dram_tensor("xhi", [XSPLIT + 1, DF], f32, kind="ExternalInput")
    mcat_d = nc.dram_tensor("mcat", [128, nt_alloc * 32], f32,
                            kind="ExternalInput")
    # int16 gather index streams in dma_gather wrapped layout: linear index
    # i = s*16 + p over [16, S], replicated 8x down the 128 partitions.
    # Linear order: block 2t = dst rows of tile t, block 2t+1 = src rows.
    glo_d = nc.dram_tensor("glo", [128, nt_alloc * 16], i16,
                           kind="ExternalInput")
    ghi_d = nc.dram_tensor("ghi", [128, nt_alloc * 16], i16,
                           kind="ExternalInput")
    loss_d = nc.dram_tensor("loss", [1, 1], f32, kind="ExternalOutput")

    with tile.TileContext(nc) as tc, \
         tc.tile_pool(name="persist", bufs=1) as pp, \
         tc.tile_pool(name="gather", bufs=1) as gp, \
         tc.tile_pool(name="work", bufs=2) as wp, \
         tc.tile_pool(name="psum", bufs=1, space="PSUM") as psp:

        mcat_sb = pp.tile([128, nt_alloc * 32], f32, tag="mcat")
        glo_sb = pp.tile([128, nt_alloc * 16], i16, tag="glo")
        ghi_sb = pp.tile([128, nt_alloc * 16], i16, tag="ghi")
        acc = pp.tile([128, nt_used], f32, tag="acc")

        nc.sync.dma_start(mcat_sb[:], mcat_d[:])
        nc.sync.dma_start(glo_sb[:], glo_d[:])
        nc.sync.dma_start(ghi_sb[:], ghi_d[:])

        # negate the maps_lo half in place: columns t*32 + i*8 + (4..7)
        m0 = mcat_sb[:]
        neg_view = AP(m0.tensor, m0.offset + 4,
                      [m0.ap[0], [32, nt_alloc], [8, D], [1, 4]])
        nc.vector.tensor_scalar(neg_view, neg_view, -1.0, None, Op.mult)

        # double-buffered gather targets: xcat[e, jj, f], jj = 0..3 dst, 4..7 src
        NIDX = 2 * GROUP * 128          # rows per gather
        SCOL = NIDX // 16               # idx columns per gather
        xg_a = gp.tile([128, GROUP * 2 * DF], f32, tag="xg_a")
        xh_a = gp.tile([128, GROUP * 2 * DF], f32, tag="xh_a")
        xg_b = gp.tile([128, GROUP * 2 * DF], f32, tag="xg_b")
        xh_b = gp.tile([128, GROUP * 2 * DF], f32, tag="xh_b")
        # static staging for the (dynamically sliced) int16 index columns
        stl_a = gp.tile([128, SCOL], i16, tag="stl_a")
        sth_a = gp.tile([128, SCOL], i16, tag="sth_a")
        stl_b = gp.tile([128, SCOL], i16, tag="stl_b")
        sth_b = gp.tile([128, SCOL], i16, tag="sth_b")

        def gather(tile0, xg, xh, stl, sth):
            # tile0: first tile index (RuntimeValue or int) of the GROUP.
            # The interleaved index stream makes the gathered rows land as
            # [x_dst | x_src] blocks per tile: row i = (2t+w)*128+p goes to
            # out[p, 2t+w, :].
            col0 = tile0 * 16
            nc.vector.tensor_copy(stl[:], glo_sb[:, ds(col0, SCOL)])
            nc.vector.tensor_copy(sth[:], ghi_sb[:, ds(col0, SCOL)])
            for xv, st, src_d in ((xg, stl, xlo_d), (xh, sth, xhi_d)):
                b = xv[:]
                out3 = AP(b.tensor, b.offset,
                          [b.ap[0], [DF, 2 * GROUP], [1, DF]])
                nc.gpsimd.dma_gather(
                    out_ap=out3, in_ap=src_d[:], idxs_ap=st[:],
                    num_idxs=NIDX, num_idxs_reg=NIDX, elem_size=DF,
                    single_packet=False)
            # merge the two half-table gathers (invalid slots gathered zeros)
            nc.vector.tensor_tensor(xg[:], xg[:], xh[:], Op.add)

        def compute(tile0, xg):
            mc_g = mcat_sb[:, ds(tile0 * 32, GROUP * 32)]
            acc_g = acc[:, ds(tile0, GROUP)]
            for k in range(GROUP):
                prod = wp.tile([128, D * 2 * DF], f32, tag="prod")
                dd = wp.tile([128, DF], f32, tag="dd")
                sq = wp.tile([128, DF], f32, tag="sq")
                xk = xg[:, 2 * DF * k:2 * DF * (k + 1)]
                # in0: xcat[e, (i) jj f] with i broadcast (stride 0)
                in0 = AP(xk.tensor, xk.offset,
                         [xk.ap[0], [0, D], [F, 2 * D], [1, F]])
                mk = mc_g[:, 32 * k:32 * (k + 1)]
                # in1: mcat[e, i jj (f)] with f broadcast (stride 0)
                in1 = AP(mk.tensor, mk.offset,
                         [mk.ap[0], [8, D], [1, 2 * D], [0, F]])
                p0 = prod[:]
                pout = AP(p0.tensor, p0.offset,
                          [p0.ap[0], [2 * DF, D], [F, 2 * D], [1, F]])
                nc.vector.tensor_tensor(pout, in0, in1, Op.mult)
                # reduce over jj (innermost): prod[e, i f jj] -> dd[e, (i f)]
                pin = AP(p0.tensor, p0.offset,
                         [p0.ap[0], [2 * DF, D], [1, F], [F, 2 * D]])
                nc.vector.tensor_reduce(dd[:], pin, axis=mybir.AxisListType.X,
                                        op=Op.add)
                nc.vector.scalar_tensor_tensor(
                    sq[:], dd[:], 0.0, dd[:], Op.bypass, Op.mult,
                    accum_out=acc_g[:, k:k + 1])

        gather(0, xg_a, xh_a, stl_a, sth_a)
        with tc.For_i(0, niters, 1,
                      hint_engines=(mybir.EngineType.DVE,)) as it:
            base = it * PAIR
            gather(base + GROUP, xg_b, xh_b, stl_b, sth_b)
            compute(base, xg_a)
            gather(base + PAIR, xg_a, xh_a, stl_a, sth_a)
            compute(base + GROUP, xg_b)

        colsum = pp.tile([128, 1], f32, tag="colsum")
        ones = pp.tile([128, 1], f32, tag="ones")
        nc.vector.reduce_sum(out=colsum[:], in_=acc[:],
                             axis=mybir.AxisListType.X)
        nc.gpsimd.memset(ones[:], 1.0)
        pt = psp.tile([1, 1], f32, tag="pt")
        nc.tensor.matmul(pt[:], lhsT=colsum[:], rhs=ones[:],
                         start=True, stop=True)
        lsb = pp.tile([1, 1], f32, tag="lsb")
        # *2: each undirected pair contributes both directed edges equally
        nc.vector.tensor_scalar(lsb[:], pt[:], 2.0, None, Op.mult)
        nc.sync.dma_start(loss_d[:], lsb[:])

    nc.compile()
    return nc


_CACHED = {}


def _get_program():
    if "nc" not in _CACHED:
        _inject_axon_hooks()
        _CACHED["nc"] = _build_program()
    return _CACHED["nc"]


def _prep_core_inputs(x_flat, maps3d, src, dst, core):
    """Build the per-core input dict (layout transforms only)."""
    e0 = core * EPC
    e1 = e0 + EPC

    # mcat rows: [e, i, jj]: jj<4 -> maps_hi[e,i,jj], jj>=4 -> maps_lo[e,i,jj-4]
    # (the maps_lo half is negated on device)
    inter = np.zeros((EPC_PAD, D, 8), np.float32)
    inter[:EPC, :, :4] = maps3d[H + e0:H + e1]
    inter[:EPC, :, 4:] = maps3d[e0:e1]
    mcat = np.zeros((128, NT_ALLOC * 32), np.float32)
    mcat[:, :NT_USED * 32] = (
        inter.reshape(NT_USED, 128, 32).transpose(1, 0, 2).reshape(128, -1))

    # linear gather order: i = (2t+w)*128 + p, w=0 dst / w=1 src
    lin = np.full((NT_ALLOC, 2, 128), XSPLIT, np.int32)
    pad = np.zeros(EPC_PAD, np.int32)
    pad[:EPC] = dst[e0:e1]
    lin[:NT_USED, 0, :] = pad.reshape(NT_USED, 128)
    pad = np.zeros(EPC_PAD, np.int32)
    pad[:EPC] = src[e0:e1]
    lin[:NT_USED, 1, :] = pad.reshape(NT_USED, 128)
    lin = lin.reshape(-1)
    lo = np.where(lin < XSPLIT, lin, XSPLIT).astype(np.int16)
    hi = np.where(lin >= XSPLIT, lin - XSPLIT, XSPLIT).astype(np.int16)
    # dma_gather wrapped layout: [16, S] with linear i = s*16 + p,
    # replicated 8x down the partitions
    glo = np.tile(lo.reshape(-1, 16).T, (8, 1))
    ghi = np.tile(hi.reshape(-1, 16).T, (8, 1))

    return {
        "mcat": np.ascontiguousarray(mcat),
        "glo": np.ascontiguousarray(glo),
        "ghi": np.ascontiguousarray(ghi),
    }


def _symmetric_structure(rev_idx):
    r = np.asarray(rev_idx)
    if r.shape != (E,):
        return False
    h = np.arange(H, dtype=r.dtype)
    return bool(np.array_equal(r[:H], h + H) and np.array_equal(r[H:], h))


def _fallback_numpy(x, restriction_maps, edge_index, rev_idx):
    x = np.asarray(x, np.float32)
    maps = np.asarray(restriction_maps, np.float32)
    ei = np.asarray(edge_index)
    rv = np.asarray(rev_idx)
    total = np.float64(0.0)
    chunk = 131072
    ne = ei.shape[1]
    for s in range(0, ne, chunk):
        e = min(s + chunk, ne)
        src = ei[0, s:e]
        tgt = ei[1, s:e]
        fvu = maps[rv[s:e]]
        fuv = maps[s:e]
        t1 = np.einsum("eij,ejf->eif", fvu, x[tgt])
        t2 = np.einsum("eij,ejf->eif", fuv, x[src])
        d = t1 - t2
        total += np.sum((d * d).astype(np.float64))
    return np.float32(total)


def kernel(x, restriction_maps, edge_index, rev_idx):
    x = np.asarray(x)
    restriction_maps = np.asarray(restriction_maps)
    edge_index = np.asarray(edge_index)
    rev_idx = np.asarray(rev_idx)

    if (x.shape != (N, D, F) or restriction_maps.shape != (E, D, D)
            or edge_index.shape != (2, E) or not _symmetric_structure(rev_idx)):
        return _fallback_numpy(x, restriction_maps, edge_index, rev_idx)

    from concourse.bass_utils import run_bass_kernel_spmd

    nc = _get_program()

    x_flat = x.reshape(N, DF).astype(np.float32)
    xlo = np.zeros((XSPLIT + 1, DF), np.float32)
    xlo[:XSPLIT] = x_flat[:XSPLIT]
    xhi = np.zeros((XSPLIT + 1, DF), np.float32)
    xhi[:N - XSPLIT] = x_flat[XSPLIT:]
    maps3d = restriction_maps.astype(np.float32)
    src = edge_index[0].astype(np.int32)
    dst = edge_index[1].astype(np.int32)

    in_maps = []
    for c in range(NCORES):
        m = _prep_core_inputs(x_flat, maps3d, src, dst, c)
        m["xlo"] = xlo
        m["xhi"] = xhi
        in_maps.append(m)
    res = run_bass_kernel_spmd(nc, in_maps, core_ids=list(range(NCORES)))
    total = np.float32(0.0)
    for c in range(NCORES):
        total += res.results[c]["loss"][0, 0]
    return np.float32(total)



# revision 3
# speedup vs baseline: 3.4890x; 3.4890x over previous
"""Trainium2 Bass kernel for sheaf Dirichlet energy (ConsistencyBasedLaplacianBuilder).

loss = sum_e || maps[rev(e)] @ x[tgt(e)] - maps[e] @ x[src(e)] ||_F^2

Strategy (edge parallelism across 8 NeuronCores):
  The edge set is symmetric (rev(e) = e +- H), so
  loss = 2 * sum_{e<H} ||maps[e+H] x[dst] - maps[e] x[src]||^2.
  Each core takes 100k half-edges.

  x is packed into bf16 pair-rows xpair[r] = [x[2r] | x[2r+1]] (256B rows)
  so a single int16-indexed dma_gather (idx = node>>1) fetches each
  endpoint; which 64-element half holds the wanted node is the node's
  parity. Edges are partitioned on the host into 4 parity classes
  (dst&1, src&1) occupying fixed tile ranges, so the parity offsets are
  compile-time constants in the access patterns.

  Per group of 8 tiles (1024 edges): one 2048-row gather (dst+src rows,
  queue round-robin over the 4 SWDGE queues so descriptor generation
  uses all Q7 core pairs), then on DVE per tile
    prod[e, i, jj, f] = mcat[e, i, jj] * xcat[e, jj, f]   (bf16)
  with mcat = [A | -B] host-prepared, followed by group-wide bf16 tree
  adds over jj (128->64->32->16 wide), and Square+accumulate on the
  Scalar engine. Per-core scalars are summed on the host.
"""

import sys
import types

import numpy as np

sys.path.insert(0, "/opt/trn_rl_repo")

N = 50000
D = 4
F = 16
DF = D * F            # 64 floats per node row
E = 1600000
H = E // 2            # 800000 undirected pairs
NCORES = 8
EPC = H // NCORES     # 100000 half-edges per core

NPAIR = N // 2 + 88   # 25088 bf16 pair rows (256B each), zero padded
GROUP = 8             # tiles per gather group
CB_G = 27             # groups per parity class
CB_EDGES = CB_G * GROUP * 128   # 26624 edge slots per class
NG = 4 * CB_G         # 104 groups per core
NT = NG * GROUP       # 832 tiles per core
NQ = 4                # SWDGE queues


def _inject_axon_hooks():
    """Provide antenv.axon_hooks if missing so NTFF tracing can register."""
    if "antenv.axon_hooks" in sys.modules:
        return
    try:
        import antenv.axon_hooks  # noqa: F401
        return
    except Exception:
        pass
    mod = types.ModuleType("antenv.axon_hooks")
    mod._hook = None

    def set_axon_ntff_profile_hook(h):
        mod._hook = h

    def get_axon_ntff_profile_hook():
        return mod._hook

    mod.set_axon_ntff_profile_hook = set_axon_ntff_profile_hook
    mod.get_axon_ntff_profile_hook = get_axon_ntff_profile_hook
    sys.modules["antenv.axon_hooks"] = mod


def _build_program():
    import concourse.bacc as bacc
    import concourse.bass as bass
    import concourse.tile as tile
    from concourse import mybir

    AP = bass.AP
    f32 = mybir.dt.float32
    bf16 = mybir.dt.bfloat16
    i16 = mybir.dt.int16
    Op = mybir.AluOpType
    Act = mybir.ActivationFunctionType
    ds = bass.ds

    nc = bacc.Bacc("TRN2", target_bir_lowering=False, debug=False,
                   num_devices=NCORES, num_swdge_queues=NQ)

    xpair_d = nc.dram_tensor("xpair", [NPAIR, 2 * DF], bf16,
                             kind="ExternalInput")
    mcat_d = nc.dram_tensor("mcat", [128, NT * 32], bf16,
                            kind="ExternalInput")
    gidx_d = nc.dram_tensor("gidx", [128, NG * 128], i16,
                            kind="ExternalInput")
    loss_d = nc.dram_tensor("loss", [1, 1], f32, kind="ExternalOutput")

    NBUF = 3

    with tile.TileContext(nc) as tc, \
         tc.tile_pool(name="persist", bufs=1) as pp, \
         tc.tile_pool(name="work", bufs=2) as wp, \
         tc.tile_pool(name="psum", bufs=1, space="PSUM") as psp:

        mcat_sb = pp.tile([128, NT * 32], bf16, tag="mcat")
        gidx_sb = pp.tile([128, NG * 128], i16, tag="gidx")
        acc = pp.tile([128, NG], f32, tag="acc")

        nc.sync.dma_start(gidx_sb[:], gidx_d[:])
        nc.sync.dma_start(mcat_sb[:], mcat_d[:])

        dbufs = [pp.tile([128, GROUP * 2 * 2 * DF], bf16, tag=f"db{i}",
                         name=f"db{i}") for i in range(NBUF)]

        def gather(g):
            db = dbufs[g % NBUF]
            b = db[:]
            out3 = AP(b.tensor, b.offset,
                      [b.ap[0], [2 * DF, 2 * GROUP], [1, 2 * DF]])
            nc.gpsimd.dma_gather(
                out_ap=out3, in_ap=xpair_d[:],
                idxs_ap=gidx_sb[:, ds(g * 128, 128)],
                num_idxs=2 * GROUP * 128, num_idxs_reg=2 * GROUP * 128,
                elem_size=2 * DF, single_packet=False, queue_num=g % NQ)

        def compute(g):
            q = g // CB_G
            pd, ps = q >> 1, q & 1
            hstride = 2 * DF + DF * (ps - pd)
            db = dbufs[g % NBUF]
            prod = wp.tile([128, GROUP * 512], bf16, tag="prod")
            t1 = wp.tile([128, GROUP * 256], bf16, tag="t1")
            t2 = wp.tile([128, GROUP * 128], bf16, tag="t2")
            dd = wp.tile([128, GROUP * 64], bf16, tag="dd")
            sq = wp.tile([128, GROUP * 64], bf16, tag="sq")

            b = db[:]
            m0 = mcat_sb[:]
            p0 = prod[:]
            for t in range(GROUP):
                in0 = AP(b.tensor, b.offset + 4 * DF * t + DF * pd,
                         [b.ap[0], [0, D], [hstride, 2], [1, DF]])
                in1 = AP(m0.tensor, m0.offset + 32 * (g * GROUP + t),
                         [m0.ap[0], [8, D], [1, 8], [0, F]])
                po = AP(p0.tensor, p0.offset + 512 * t,
                        [p0.ap[0], [128, D], [DF, 2], [1, DF]])
                nc.vector.tensor_tensor(po, in0, in1, Op.mult)

            # tree-reduce over jj: per (tile,i) 128-block: h halves, then jl
            a0 = AP(p0.tensor, p0.offset, [p0.ap[0], [128, 32], [1, 64]])
            a1 = AP(p0.tensor, p0.offset + 64, [p0.ap[0], [128, 32], [1, 64]])
            t1v = t1[:]
            o1 = AP(t1v.tensor, t1v.offset, [t1v.ap[0], [64, 32], [1, 64]])
            nc.vector.tensor_tensor(o1, a0, a1, Op.add)

            b0 = AP(t1v.tensor, t1v.offset, [t1v.ap[0], [64, 32], [1, 32]])
            b1 = AP(t1v.tensor, t1v.offset + 32,
                    [t1v.ap[0], [64, 32], [1, 32]])
            t2v = t2[:]
            o2 = AP(t2v.tensor, t2v.offset, [t2v.ap[0], [32, 32], [1, 32]])
            nc.vector.tensor_tensor(o2, b0, b1, Op.add)

            c0 = AP(t2v.tensor, t2v.offset, [t2v.ap[0], [32, 32], [1, 16]])
            c1 = AP(t2v.tensor, t2v.offset + 16,
                    [t2v.ap[0], [32, 32], [1, 16]])
            ddv = dd[:]
            o3 = AP(ddv.tensor, ddv.offset, [ddv.ap[0], [16, 32], [1, 16]])
            nc.vector.tensor_tensor(o3, c0, c1, Op.add)

            nc.scalar.activation(sq[:], dd[:], Act.Square,
                                 accum_out=acc[:, g:g + 1])

        for g in range(NG):
            gather(g)
            if g >= 1:
                compute(g - 1)
        compute(NG - 1)

        colsum = pp.tile([128, 1], f32, tag="colsum")
        ones = pp.tile([128, 1], f32, tag="ones")
        nc.vector.reduce_sum(out=colsum[:], in_=acc[:],
                             axis=mybir.AxisListType.X)
        nc.vector.memset(ones[:], 1.0)
        pt = psp.tile([1, 1], f32, tag="pt")
        nc.tensor.matmul(pt[:], lhsT=colsum[:], rhs=ones[:],
                         start=True, stop=True)
        lsb = pp.tile([1, 1], f32, tag="lsb")
        # *2: each undirected pair contributes both directed edges equally
        nc.vector.tensor_scalar(lsb[:], pt[:], 2.0, None, Op.mult)
        nc.sync.dma_start(loss_d[:], lsb[:])

    nc.compile()
    return nc


_CACHED = {}


def _get_program():
    if "nc" not in _CACHED:
        _inject_axon_hooks()
        _CACHED["nc"] = _build_program()
    return _CACHED["nc"]


def _bf16(a):
    import ml_dtypes
    return a.astype(ml_dtypes.bfloat16)


def _prep_core_inputs(maps3d, src, dst, core):
    """Per-core layout transforms. Returns dict or None if class overflow."""
    e0 = core * EPC
    e1 = e0 + EPC
    d = dst[e0:e1]
    s = src[e0:e1]
    A = maps3d[H + e0:H + e1]
    B = maps3d[e0:e1]

    cls = (d & 1) * 2 + (s & 1)
    eidx = np.full(NT * 128, -1, np.int64)
    for q in range(4):
        iq = np.flatnonzero(cls == q)
        if len(iq) > CB_EDGES:
            return None
        eidx[q * CB_EDGES:q * CB_EDGES + len(iq)] = iq
    valid = eidx >= 0
    ev = eidx[valid]

    m8 = np.zeros((NT * 128, D, 8), np.float32)
    m8[valid, :, :4] = A[ev]
    m8[valid, :, 4:] = -B[ev]
    mcat = _bf16(m8.reshape(NT, 128, 32).transpose(1, 0, 2)
                 .reshape(128, NT * 32))

    dstP = np.zeros(NT * 128, np.int64)
    dstP[valid] = d[ev]
    srcP = np.zeros(NT * 128, np.int64)
    srcP[valid] = s[ev]
    lin = np.empty((NT, 2, 128), np.int16)
    lin[:, 0, :] = (dstP >> 1).reshape(NT, 128)
    lin[:, 1, :] = (srcP >> 1).reshape(NT, 128)
    gidx = np.tile(lin.reshape(-1, 16).T, (8, 1))

    return {
        "mcat": np.ascontiguousarray(mcat),
        "gidx": np.ascontiguousarray(gidx),
    }


def _make_in_maps(x, restriction_maps, edge_index):
    """Build per-core input maps (shared xpair included). None on overflow."""
    x_flat = x.reshape(N, DF).astype(np.float32)
    xp = np.zeros((NPAIR, 2 * DF), np.float32)
    xp[:N // 2, :DF] = x_flat[0::2]
    xp[:N // 2, DF:] = x_flat[1::2]
    xpair = _bf16(xp)
    maps3d = restriction_maps.astype(np.float32)
    src = edge_index[0].astype(np.int64)
    dst = edge_index[1].astype(np.int64)
    in_maps = []
    for c in range(NCORES):
        m = _prep_core_inputs(maps3d, src, dst, c)
        if m is None:
            return None
        m["xpair"] = xpair
        in_maps.append(m)
    return in_maps


def _symmetric_structure(rev_idx):
    r = np.asarray(rev_idx)
    if r.shape != (E,):
        return False
    h = np.arange(H, dtype=r.dtype)
    return bool(np.array_equal(r[:H], h + H) and np.array_equal(r[H:], h))


def _fallback_numpy(x, restriction_maps, edge_index, rev_idx):
    x = np.asarray(x, np.float32)
    maps = np.asarray(restriction_maps, np.float32)
    ei = np.asarray(edge_index)
    rv = np.asarray(rev_idx)
    total = np.float64(0.0)
    chunk = 131072
    ne = ei.shape[1]
    for st in range(0, ne, chunk):
        e = min(st + chunk, ne)
        srcc = ei[0, st:e]
        tgt = ei[1, st:e]
        fvu = maps[rv[st:e]]
        fuv = maps[st:e]
        t1 = np.einsum("eij,ejf->eif", fvu, x[tgt])
        t2 = np.einsum("eij,ejf->eif", fuv, x[srcc])
        dd = t1 - t2
        total += np.sum((dd * dd).astype(np.float64))
    return np.float32(total)


def kernel(x, restriction_maps, edge_index, rev_idx):
    x = np.asarray(x)
    restriction_maps = np.asarray(restriction_maps)
    edge_index = np.asarray(edge_index)
    rev_idx = np.asarray(rev_idx)

    if (x.shape != (N, D, F) or restriction_maps.shape != (E, D, D)
            or edge_index.shape != (2, E) or not _symmetric_structure(rev_idx)):
        return _fallback_numpy(x, restriction_maps, edge_index, rev_idx)

    in_maps = _make_in_maps(x, restriction_maps, edge_index)
    if in_maps is None:
        return _fallback_numpy(x, restriction_maps, edge_index, rev_idx)

    from concourse.bass_utils import run_bass_kernel_spmd

    nc = _get_program()
    res = run_bass_kernel_spmd(nc, in_maps, core_ids=list(range(NCORES)))
    total = np.float32(0.0)
    for c in range(NCORES):
        total += res.results[c]["loss"][0, 0]
    return np.float32(total)


# revision 4
# speedup vs baseline: 4.9417x; 1.4164x over previous
"""Trainium2 Bass kernel for sheaf Dirichlet energy (ConsistencyBasedLaplacianBuilder).

loss = sum_e || maps[rev(e)] @ x[tgt(e)] - maps[e] @ x[src(e)] ||_F^2

Strategy (edge parallelism across 8 NeuronCores):
  The edge set is symmetric (rev(e) = e +- H), so
  loss = 2 * sum_{e<H} ||maps[e+H] x[dst] - maps[e] x[src]||^2.
  Each core takes 100k half-edges.

  x is packed into bf16 pair-rows xpair[r] = [x[2r] | x[2r+1]] (256B rows)
  so a single int16-indexed dma_gather (idx = node>>1) fetches each
  endpoint; which 64-element half holds the wanted node is the node's
  parity. Edges are partitioned on the host into 4 parity classes
  (dst&1, src&1) occupying fixed tile ranges, so the parity offsets are
  compile-time constants in the access patterns.

  Per group of 8 tiles (1024 edges): one 2048-row gather (dst+src rows,
  queue round-robin over the 4 SWDGE queues so descriptor generation
  uses all Q7 core pairs), then on DVE per tile
    prod[e, i, jj, f] = mcat[e, i, jj] * xcat[e, jj, f]   (bf16)
  with mcat = [A | -B] host-prepared, followed by group-wide bf16 tree
  adds over jj (128->64->32->16 wide), and Square+accumulate on the
  Scalar engine. Per-core scalars are summed on the host.
"""

import sys
import types

import numpy as np

sys.path.insert(0, "/opt/trn_rl_repo")

N = 50000
D = 4
F = 16
DF = D * F            # 64 floats per node row
E = 1600000
H = E // 2            # 800000 undirected pairs
NCORES = 8
EPC = H // NCORES     # 100000 half-edges per core

NPAIR = N // 2 + 88   # 25088 bf16 pair rows (256B each), zero padded
GROUP = 8             # tiles per gather group
CB_G = 27             # groups per parity class
CB_EDGES = CB_G * GROUP * 128   # 26624 edge slots per class
NG = 4 * CB_G         # 104 groups per core
NT = NG * GROUP       # 832 tiles per core
NQ = 4                # SWDGE queues


def _inject_axon_hooks():
    """Provide antenv.axon_hooks if missing so NTFF tracing can register."""
    if "antenv.axon_hooks" in sys.modules:
        return
    try:
        import antenv.axon_hooks  # noqa: F401
        return
    except Exception:
        pass
    mod = types.ModuleType("antenv.axon_hooks")
    mod._hook = None

    def set_axon_ntff_profile_hook(h):
        mod._hook = h

    def get_axon_ntff_profile_hook():
        return mod._hook

    mod.set_axon_ntff_profile_hook = set_axon_ntff_profile_hook
    mod.get_axon_ntff_profile_hook = get_axon_ntff_profile_hook
    sys.modules["antenv.axon_hooks"] = mod


def _build_program():
    import concourse.bacc as bacc
    import concourse.bass as bass
    import concourse.tile as tile
    from concourse import mybir

    AP = bass.AP
    f32 = mybir.dt.float32
    bf16 = mybir.dt.bfloat16
    i16 = mybir.dt.int16
    Op = mybir.AluOpType
    Act = mybir.ActivationFunctionType
    ds = bass.ds

    nc = bacc.Bacc("TRN2", target_bir_lowering=False, debug=False,
                   num_devices=NCORES, num_swdge_queues=NQ)

    xpair_d = nc.dram_tensor("xpair", [NPAIR, 2 * DF], bf16,
                             kind="ExternalInput")
    mcat_d = nc.dram_tensor("mcat", [128, NT * 32], bf16,
                            kind="ExternalInput")
    gidx_d = nc.dram_tensor("gidx", [128, NG * 128], i16,
                            kind="ExternalInput")
    loss_d = nc.dram_tensor("loss", [1, 1], f32, kind="ExternalOutput")

    NBUF = 12

    with tile.TileContext(nc) as tc, \
         tc.tile_pool(name="persist", bufs=1) as pp, \
         tc.tile_pool(name="work", bufs=2) as wp, \
         tc.tile_pool(name="psum", bufs=1, space="PSUM") as psp:

        mcat_sb = pp.tile([128, NT * 32], bf16, tag="mcat")
        gidx_sb = pp.tile([128, NG * 128], i16, tag="gidx")
        acc = pp.tile([128, NG], f32, tag="acc")

        nc.sync.dma_start(gidx_sb[:], gidx_d[:])
        nc.sync.dma_start(mcat_sb[:], mcat_d[:])

        dbufs = [pp.tile([128, GROUP * 2 * 2 * DF], bf16, tag=f"db{i}",
                         name=f"db{i}") for i in range(NBUF)]

        def gather(g):
            db = dbufs[g % NBUF]
            b = db[:]
            out3 = AP(b.tensor, b.offset,
                      [b.ap[0], [2 * DF, 2 * GROUP], [1, 2 * DF]])
            nc.gpsimd.dma_gather(
                out_ap=out3, in_ap=xpair_d[:],
                idxs_ap=gidx_sb[:, ds(g * 128, 128)],
                num_idxs=2 * GROUP * 128, num_idxs_reg=2 * GROUP * 128,
                elem_size=2 * DF, single_packet=False, queue_num=g % NQ)

        def compute(g):
            q = g // CB_G
            pd, ps = q >> 1, q & 1
            hstride = 2 * DF + DF * (ps - pd)
            db = dbufs[g % NBUF]
            prod = wp.tile([128, GROUP * 512], bf16, tag="prod")
            t1 = wp.tile([128, GROUP * 256], bf16, tag="t1")
            t2 = wp.tile([128, GROUP * 128], bf16, tag="t2")
            dd = wp.tile([128, GROUP * 64], bf16, tag="dd")
            sq = wp.tile([128, GROUP * 64], bf16, tag="sq")

            b = db[:]
            m0 = mcat_sb[:]
            p0 = prod[:]
            for t in range(GROUP):
                in0 = AP(b.tensor, b.offset + 4 * DF * t + DF * pd,
                         [b.ap[0], [0, D], [hstride, 2], [1, DF]])
                in1 = AP(m0.tensor, m0.offset + 32 * (g * GROUP + t),
                         [m0.ap[0], [8, D], [1, 8], [0, F]])
                po = AP(p0.tensor, p0.offset + 512 * t,
                        [p0.ap[0], [128, D], [DF, 2], [1, DF]])
                nc.vector.tensor_tensor(po, in0, in1, Op.mult)

            # tree-reduce over jj: per (tile,i) 128-block: h halves, then jl
            a0 = AP(p0.tensor, p0.offset, [p0.ap[0], [128, 32], [1, 64]])
            a1 = AP(p0.tensor, p0.offset + 64, [p0.ap[0], [128, 32], [1, 64]])
            t1v = t1[:]
            o1 = AP(t1v.tensor, t1v.offset, [t1v.ap[0], [64, 32], [1, 64]])
            nc.vector.tensor_tensor(o1, a0, a1, Op.add)

            b0 = AP(t1v.tensor, t1v.offset, [t1v.ap[0], [64, 32], [1, 32]])
            b1 = AP(t1v.tensor, t1v.offset + 32,
                    [t1v.ap[0], [64, 32], [1, 32]])
            t2v = t2[:]
            o2 = AP(t2v.tensor, t2v.offset, [t2v.ap[0], [32, 32], [1, 32]])
            nc.vector.tensor_tensor(o2, b0, b1, Op.add)

            c0 = AP(t2v.tensor, t2v.offset, [t2v.ap[0], [32, 32], [1, 16]])
            c1 = AP(t2v.tensor, t2v.offset + 16,
                    [t2v.ap[0], [32, 32], [1, 16]])
            ddv = dd[:]
            o3 = AP(ddv.tensor, ddv.offset, [ddv.ap[0], [16, 32], [1, 16]])
            nc.vector.tensor_tensor(o3, c0, c1, Op.add)

            nc.scalar.activation(sq[:], dd[:], Act.Square,
                                 accum_out=acc[:, g:g + 1])

        for g in range(NG):
            gather(g)
            if g >= 1:
                compute(g - 1)
        compute(NG - 1)

        colsum = pp.tile([128, 1], f32, tag="colsum")
        ones = pp.tile([128, 1], f32, tag="ones")
        nc.vector.reduce_sum(out=colsum[:], in_=acc[:],
                             axis=mybir.AxisListType.X)
        nc.vector.memset(ones[:], 1.0)
        pt = psp.tile([1, 1], f32, tag="pt")
        nc.tensor.matmul(pt[:], lhsT=colsum[:], rhs=ones[:],
                         start=True, stop=True)
        lsb = pp.tile([1, 1], f32, tag="lsb")
        # *2: each undirected pair contributes both directed edges equally
        nc.vector.tensor_scalar(lsb[:], pt[:], 2.0, None, Op.mult)
        nc.sync.dma_start(loss_d[:], lsb[:])

    nc.compile()
    return nc


_CACHED = {}


def _get_program():
    if "nc" not in _CACHED:
        _inject_axon_hooks()
        _CACHED["nc"] = _build_program()
    return _CACHED["nc"]


def _bf16(a):
    import ml_dtypes
    return a.astype(ml_dtypes.bfloat16)


def _prep_core_inputs(maps3d, src, dst, core):
    """Per-core layout transforms. Returns dict or None if class overflow."""
    e0 = core * EPC
    e1 = e0 + EPC
    d = dst[e0:e1]
    s = src[e0:e1]
    A = maps3d[H + e0:H + e1]
    B = maps3d[e0:e1]

    cls = (d & 1) * 2 + (s & 1)
    eidx = np.full(NT * 128, -1, np.int64)
    for q in range(4):
        iq = np.flatnonzero(cls == q)
        if len(iq) > CB_EDGES:
            return None
        eidx[q * CB_EDGES:q * CB_EDGES + len(iq)] = iq
    valid = eidx >= 0
    ev = eidx[valid]

    m8 = np.zeros((NT * 128, D, 8), np.float32)
    m8[valid, :, :4] = A[ev]
    m8[valid, :, 4:] = -B[ev]
    mcat = _bf16(m8.reshape(NT, 128, 32).transpose(1, 0, 2)
                 .reshape(128, NT * 32))

    dstP = np.zeros(NT * 128, np.int64)
    dstP[valid] = d[ev]
    srcP = np.zeros(NT * 128, np.int64)
    srcP[valid] = s[ev]
    lin = np.empty((NT, 2, 128), np.int16)
    lin[:, 0, :] = (dstP >> 1).reshape(NT, 128)
    lin[:, 1, :] = (srcP >> 1).reshape(NT, 128)
    gidx = np.tile(lin.reshape(-1, 16).T, (8, 1))

    return {
        "mcat": np.ascontiguousarray(mcat),
        "gidx": np.ascontiguousarray(gidx),
    }


def _make_in_maps(x, restriction_maps, edge_index):
    """Build per-core input maps (shared xpair included). None on overflow."""
    x_flat = x.reshape(N, DF).astype(np.float32)
    xp = np.zeros((NPAIR, 2 * DF), np.float32)
    xp[:N // 2, :DF] = x_flat[0::2]
    xp[:N // 2, DF:] = x_flat[1::2]
    xpair = _bf16(xp)
    maps3d = restriction_maps.astype(np.float32)
    src = edge_index[0].astype(np.int64)
    dst = edge_index[1].astype(np.int64)
    in_maps = []
    for c in range(NCORES):
        m = _prep_core_inputs(maps3d, src, dst, c)
        if m is None:
            return None
        m["xpair"] = xpair
        in_maps.append(m)
    return in_maps


def _symmetric_structure(rev_idx):
    r = np.asarray(rev_idx)
    if r.shape != (E,):
        return False
    h = np.arange(H, dtype=r.dtype)
    return bool(np.array_equal(r[:H], h + H) and np.array_equal(r[H:], h))


def _fallback_numpy(x, restriction_maps, edge_index, rev_idx):
    x = np.asarray(x, np.float32)
    maps = np.asarray(restriction_maps, np.float32)
    ei = np.asarray(edge_index)
    rv = np.asarray(rev_idx)
    total = np.float64(0.0)
    chunk = 131072
    ne = ei.shape[1]
    for st in range(0, ne, chunk):
        e = min(st + chunk, ne)
        srcc = ei[0, st:e]
        tgt = ei[1, st:e]
        fvu = maps[rv[st:e]]
        fuv = maps[st:e]
        t1 = np.einsum("eij,ejf->eif", fvu, x[tgt])
        t2 = np.einsum("eij,ejf->eif", fuv, x[srcc])
        dd = t1 - t2
        total += np.sum((dd * dd).astype(np.float64))
    return np.float32(total)


def kernel(x, restriction_maps, edge_index, rev_idx):
    x = np.asarray(x)
    restriction_maps = np.asarray(restriction_maps)
    edge_index = np.asarray(edge_index)
    rev_idx = np.asarray(rev_idx)

    if (x.shape != (N, D, F) or restriction_maps.shape != (E, D, D)
            or edge_index.shape != (2, E) or not _symmetric_structure(rev_idx)):
        return _fallback_numpy(x, restriction_maps, edge_index, rev_idx)

    in_maps = _make_in_maps(x, restriction_maps, edge_index)
    if in_maps is None:
        return _fallback_numpy(x, restriction_maps, edge_index, rev_idx)

    from concourse.bass_utils import run_bass_kernel_spmd

    nc = _get_program()
    res = run_bass_kernel_spmd(nc, in_maps, core_ids=list(range(NCORES)))
    total = np.float32(0.0)
    for c in range(NCORES):
        total += res.results[c]["loss"][0, 0]
    return np.float32(total)
